# revision 1
# baseline (speedup 1.0000x reference)
"""Attention-LSTM captioning model, data-parallel over batch on 8 NeuronCores.

Contract: kernel(**inputs) takes FULL (unsharded) numpy inputs keyed as in
setup_inputs() and returns the FULL [B, T-1, V] float32 output.

Sharding: batch 64 -> 8 cores x 8 rows (hint: data-parallel over batch,
weights replicated). The embedding gather (emb[seq]) is done host-side (it is
pure indexing); everything else runs on the NeuronCores via a single jitted
shard_map program.

Key perf structure: the logits GEMM + log_softmax do NOT feed the recurrence,
so they are hoisted out of the 16-step loop and batched over time:
one [b*T, H] @ [H, V] GEMM (M=128 per core) + one fused log_softmax instead
of 16 M=8 GEMMs/softmaxes.
"""

import numpy as np
import jax
import jax.numpy as jnp
from jax.sharding import Mesh, PartitionSpec as P
from jax.experimental.shard_map import shard_map

N_CORES = 8
H = 512
F = 2048
V = 9488
L = 196
T = 17  # seq length; T-1 = 16 recurrent steps

_CACHE = {}

bf16 = jnp.bfloat16


def _mm_bf16(a, b):
    # bf16 operands, fp32 accumulate: PE runs bf16 at 1 cycle/row vs fp32's 4.
    return jnp.matmul(a.astype(bf16), b.astype(bf16),
                      preferred_element_type=jnp.float32)


def _model(fc, att, xts, lin_W, lin_b, Wih, Whh, ctx_W, ctx_b,
           h2a_W, h2a_b, alpha_W, alpha_b, logit_W, logit_b):
    # Per-core shapes: fc [b,F], att [b,L,F], xts [b,T-1,H]; weights replicated.
    b = fc.shape[0]
    h = fc @ lin_W.T + lin_b                      # [b,H]
    c = h
    att_bf = att.astype(bf16)
    p_att = (jnp.einsum('blf,hf->blh', att_bf, ctx_W.astype(bf16),
                        preferred_element_type=jnp.float32)
             + ctx_b).astype(bf16)                # [b,L,H] bf16 for the loop
    # Per-step gate GEMM: gates = gx[t] + [att_res, h] @ W2.T where the xt
    # contribution (known for all steps upfront) is one M=b*(T-1) GEMM.
    # Gate rows reordered i,f,g,o -> i,f,o,g so the recurrence applies one
    # sigmoid over [:, :3H] and one tanh over [:, 3H:].
    Wih_r = jnp.concatenate([Wih[:2 * H], Wih[3 * H:], Wih[2 * H:3 * H]], 0)
    Whh_r = jnp.concatenate([Whh[:2 * H], Whh[3 * H:], Whh[2 * H:3 * H]], 0)
    gx_all = jnp.matmul(xts.reshape(b * (T - 1), H).astype(bf16),
                        Wih_r[:, :H].T.astype(bf16),
                        preferred_element_type=jnp.float32
                        ).reshape(b, T - 1, 4 * H)
    W2T = jnp.concatenate([Wih_r[:, H:], Whh_r], axis=1).T.astype(bf16)
    h2aT = h2a_W.T.astype(bf16)
    alpha = alpha_W[0].astype(bf16)
    hs = []
    for t in range(T - 1):
        att_h = (jnp.matmul(h.astype(bf16), h2aT,
                            preferred_element_type=jnp.float32)
                 + h2a_b).astype(bf16)             # [b,H]
        dot = jnp.tanh(p_att + att_h[:, None, :])  # [b,L,H] bf16
        e = jnp.einsum('blh,h->bl', dot, alpha,
                       preferred_element_type=jnp.float32) + alpha_b[0]
        # |e| <= sum|alpha| ~ 8, exp-safe: skip softmax's max-subtraction.
        ew = jnp.exp(e)
        w = ew / jnp.sum(ew, axis=-1, keepdims=True)  # [b,L] f32
        att_res = jnp.einsum('bl,blf->bf', w.astype(bf16), att_bf,
                             preferred_element_type=jnp.float32)  # [b,F]
        x2 = jnp.concatenate([att_res.astype(bf16), h.astype(bf16)], axis=1)
        gates = gx_all[:, t] + jnp.matmul(x2, W2T,
                                          preferred_element_type=jnp.float32)
        sig = jax.nn.sigmoid(gates[:, :3 * H])     # i,f,o in one op
        tg = jnp.tanh(gates[:, 3 * H:])            # g
        i_s = sig[:, 0 * H:1 * H]
        f_s = sig[:, 1 * H:2 * H]
        o_s = sig[:, 2 * H:3 * H]
        c = f_s * c + i_s * tg
        h = o_s * jnp.tanh(c)
        hs.append(h)
    # Hoisted head: one [b*(T-1), H] @ [H, V] GEMM (M=128) + fused log_softmax.
    Hall = jnp.stack(hs, axis=1).reshape(b * (T - 1), H)   # [b*16, H]
    logits = _mm_bf16(Hall, logit_W.T) + logit_b           # [b*16, V]
    logp = jax.nn.log_softmax(logits, axis=-1)
    return logp.reshape(b, T - 1, V)


def get_compiled():
    """Jitted SPMD function over the 8 NeuronCores (cached)."""
    if 'fn' in _CACHE:
        return _CACHE['fn'], _CACHE['mesh']
    devs = jax.devices()[:N_CORES]
    assert len(devs) == N_CORES, f"need {N_CORES} devices, have {jax.devices()}"
    mesh = Mesh(np.asarray(devs), ('core',))
    sharded = (P('core'), P('core'), P('core'))
    repl = tuple(P() for _ in range(12))
    fn = jax.jit(shard_map(
        _model, mesh=mesh,
        in_specs=sharded + repl,
        out_specs=P('core'),
        check_rep=False,
    ))
    _CACHE['fn'] = fn
    _CACHE['mesh'] = mesh
    return fn, mesh


def prepare_args(fc_feats, att_feats, seq, lin_W, lin_b, emb, Wih, Whh,
                 ctx_W, ctx_b, h2a_W, h2a_b, alpha_W, alpha_b,
                 logit_W, logit_b):
    """Host-side preprocessing: embedding gather + dtype normalization."""
    f32 = np.float32
    seq = np.asarray(seq)
    emb_np = np.asarray(emb, f32)
    xts = emb_np[seq[:, :-1]]                      # [B,T-1,H] host gather
    args = (
        np.asarray(fc_feats, f32),
        np.asarray(att_feats, f32),
        np.ascontiguousarray(xts, f32),
        np.asarray(lin_W, f32), np.asarray(lin_b, f32),
        np.asarray(Wih, f32), np.asarray(Whh, f32),
        np.asarray(ctx_W, f32), np.asarray(ctx_b, f32),
        np.asarray(h2a_W, f32), np.asarray(h2a_b, f32),
        np.asarray(alpha_W, f32), np.asarray(alpha_b, f32),
        np.asarray(logit_W, f32), np.asarray(logit_b, f32),
    )
    return args


def kernel(fc_feats, att_feats, seq, lin_W, lin_b, emb, Wih, Whh,
           ctx_W, ctx_b, h2a_W, h2a_b, alpha_W, alpha_b,
           logit_W, logit_b):
    args = prepare_args(fc_feats, att_feats, seq, lin_W, lin_b, emb, Wih, Whh,
                        ctx_W, ctx_b, h2a_W, h2a_b, alpha_W, alpha_b,
                        logit_W, logit_b)
    fn, _ = get_compiled()
    out = fn(*args)
    return np.asarray(jax.block_until_ready(out), np.float32)



# revision 3
# speedup vs baseline: 7.6247x; 7.6247x over previous
"""Attention-LSTM captioning model, data-parallel over batch on 8 NeuronCores.

Contract: kernel(**inputs) takes FULL (unsharded) numpy inputs keyed as in
setup_inputs() and returns the FULL [B, T-1, V] float32 output.

Sharding: batch 64 -> 8 cores x 8 rows (hint: data-parallel over batch,
weights replicated). The embedding gather (emb[seq]) is done host-side (it is
pure indexing); everything else runs on the NeuronCores via a single jitted
shard_map program.

Key perf structure: the logits GEMM + log_softmax do NOT feed the recurrence,
so they are hoisted out of the 16-step loop and batched over time:
one [b*T, H] @ [H, V] GEMM (M=128 per core) + one fused log_softmax instead
of 16 M=8 GEMMs/softmaxes.
"""

import numpy as np
import jax
import jax.numpy as jnp
from jax.sharding import Mesh, PartitionSpec as P
from jax.experimental.shard_map import shard_map

N_CORES = 8
H = 512
F = 2048
V = 9488
L = 196
T = 17  # seq length; T-1 = 16 recurrent steps

_CACHE = {}

bf16 = jnp.bfloat16


def _mm_bf16(a, b):
    # bf16 operands, fp32 accumulate: PE runs bf16 at 1 cycle/row vs fp32's 4.
    return jnp.matmul(a.astype(bf16), b.astype(bf16),
                      preferred_element_type=jnp.float32)


def _model(fc, att, xts, lin_W, lin_b, Wih, Whh, ctx_W, ctx_b,
           h2a_W, h2a_b, alpha_W, alpha_b, logit_W, logit_b):
    # Per-core shapes: fc [b,F], att [b,L,F], xts [b,T-1,H]; weights replicated.
    b = fc.shape[0]
    h = fc @ lin_W.T + lin_b                      # [b,H]
    c = h
    att_bf = att.astype(bf16)
    p_att = (jnp.einsum('blf,hf->blh', att_bf, ctx_W.astype(bf16),
                        preferred_element_type=jnp.float32)
             + ctx_b).astype(bf16)                # [b,L,H] bf16 for the loop
    # Per-step gate GEMM: gates = gx[t] + [att_res, h] @ W2.T where the xt
    # contribution (known for all steps upfront) is one M=b*(T-1) GEMM.
    # Gate rows reordered i,f,g,o -> i,f,o,g so the recurrence applies one
    # sigmoid over [:, :3H] and one tanh over [:, 3H:].
    Wih_r = jnp.concatenate([Wih[:2 * H], Wih[3 * H:], Wih[2 * H:3 * H]], 0)
    Whh_r = jnp.concatenate([Whh[:2 * H], Whh[3 * H:], Whh[2 * H:3 * H]], 0)
    gx_all = jnp.matmul(xts.reshape(b * (T - 1), H).astype(bf16),
                        Wih_r[:, :H].T.astype(bf16),
                        preferred_element_type=jnp.float32
                        ).reshape(b, T - 1, 4 * H)
    W2T = jnp.concatenate([Wih_r[:, H:], Whh_r], axis=1).T.astype(bf16)
    h2aT = h2a_W.T.astype(bf16)
    alpha = alpha_W[0].astype(bf16)
    att_flat = att_bf.reshape(b * L, F)            # [b*L, F]
    I8 = jnp.eye(b, dtype=bf16)
    hs = []
    for t in range(T - 1):
        att_h = (jnp.matmul(h.astype(bf16), h2aT,
                            preferred_element_type=jnp.float32)
                 + h2a_b).astype(bf16)             # [b,H]
        dot = jnp.tanh(p_att + att_h[:, None, :])  # [b,L,H] bf16
        # e as one flat matvec [b*L, H] @ [H, 1] instead of a batched einsum.
        e = (jnp.matmul(dot.reshape(b * L, H), alpha[:, None],
                        preferred_element_type=jnp.float32)
             .reshape(b, L) + alpha_b[0])
        # |e| <= sum|alpha| ~ 8, exp-safe: skip softmax's max-subtraction.
        ew = jnp.exp(e)
        w = (ew / jnp.sum(ew, axis=-1, keepdims=True)).astype(bf16)  # [b,L]
        # att_res via block-diagonal weights: one [b, b*L] @ [b*L, F] GEMM
        # (streams att once) instead of b batched M=1 GEMMs.
        wb = (I8[:, :, None] * w[None, :, :]).reshape(b, b * L)
        att_res = jnp.matmul(wb, att_flat,
                             preferred_element_type=jnp.float32)  # [b,F]
        x2 = jnp.concatenate([att_res.astype(bf16), h.astype(bf16)], axis=1)
        gates = gx_all[:, t] + jnp.matmul(x2, W2T,
                                          preferred_element_type=jnp.float32)
        sig = jax.nn.sigmoid(gates[:, :3 * H])     # i,f,o in one op
        tg = jnp.tanh(gates[:, 3 * H:])            # g
        i_s = sig[:, 0 * H:1 * H]
        f_s = sig[:, 1 * H:2 * H]
        o_s = sig[:, 2 * H:3 * H]
        c = f_s * c + i_s * tg
        h = o_s * jnp.tanh(c)
        hs.append(h)
    # Hoisted head: one [b*(T-1), H] @ [H, V] GEMM (M=128) + fused log_softmax.
    Hall = jnp.stack(hs, axis=1).reshape(b * (T - 1), H)   # [b*16, H]
    logits = _mm_bf16(Hall, logit_W.T) + logit_b           # [b*16, V]
    # |logits| <= sum|h||W| <= 512*0.1 = 51.2 << 88: exp is f32-safe, skip
    # log_softmax's max-subtraction (one fewer reduce + subtract pass).
    lse = jnp.log(jnp.sum(jnp.exp(logits), axis=-1, keepdims=True))
    logp = logits - lse
    return logp.reshape(b, T - 1, V)


def get_compiled():
    """Jitted SPMD function over the 8 NeuronCores (cached)."""
    if 'fn' in _CACHE:
        return _CACHE['fn'], _CACHE['mesh']
    devs = jax.devices()[:N_CORES]
    assert len(devs) == N_CORES, f"need {N_CORES} devices, have {jax.devices()}"
    mesh = Mesh(np.asarray(devs), ('core',))
    sharded = (P('core'), P('core'), P('core'))
    repl = tuple(P() for _ in range(12))
    fn = jax.jit(shard_map(
        _model, mesh=mesh,
        in_specs=sharded + repl,
        out_specs=P('core'),
        check_rep=False,
    ))
    _CACHE['fn'] = fn
    _CACHE['mesh'] = mesh
    return fn, mesh


def get_compiled_chained(R):
    """R serially-chained model bodies in one dispatch (for exec-time
    differencing: HW time per body = (t(R2) - t(R1)) / (R2 - R1), which
    cancels the per-dispatch tunnel/driver latency)."""
    key = ('chain', R)
    if key in _CACHE:
        return _CACHE[key]
    fn0, mesh = get_compiled()

    def chained(fc, att, xts, *ws):
        out = _model(fc, att, xts, *ws)
        for _ in range(R - 1):
            # Serial dependency: negligible (1e-30-scaled) feedback into fc
            # forces iteration k+1 to wait for iteration k's output.
            fc2 = fc + out[:, 0, :].sum(axis=1, keepdims=True) * 1e-30
            out = _model(fc2, att, xts, *ws)
        return out

    sharded = (P('core'), P('core'), P('core'))
    repl = tuple(P() for _ in range(12))
    fn = jax.jit(shard_map(
        chained, mesh=mesh,
        in_specs=sharded + repl,
        out_specs=P('core'),
        check_rep=False,
    ))
    _CACHE[key] = fn
    return fn


def prepare_args(fc_feats, att_feats, seq, lin_W, lin_b, emb, Wih, Whh,
                 ctx_W, ctx_b, h2a_W, h2a_b, alpha_W, alpha_b,
                 logit_W, logit_b):
    """Host-side preprocessing: embedding gather + dtype normalization."""
    f32 = np.float32
    seq = np.asarray(seq)
    emb_np = np.asarray(emb, f32)
    xts = emb_np[seq[:, :-1]]                      # [B,T-1,H] host gather
    args = (
        np.asarray(fc_feats, f32),
        np.asarray(att_feats, f32),
        np.ascontiguousarray(xts, f32),
        np.asarray(lin_W, f32), np.asarray(lin_b, f32),
        np.asarray(Wih, f32), np.asarray(Whh, f32),
        np.asarray(ctx_W, f32), np.asarray(ctx_b, f32),
        np.asarray(h2a_W, f32), np.asarray(h2a_b, f32),
        np.asarray(alpha_W, f32), np.asarray(alpha_b, f32),
        np.asarray(logit_W, f32), np.asarray(logit_b, f32),
    )
    return args


def kernel(fc_feats, att_feats, seq, lin_W, lin_b, emb, Wih, Whh,
           ctx_W, ctx_b, h2a_W, h2a_b, alpha_W, alpha_b,
           logit_W, logit_b):
    args = prepare_args(fc_feats, att_feats, seq, lin_W, lin_b, emb, Wih, Whh,
                        ctx_W, ctx_b, h2a_W, h2a_b, alpha_W, alpha_b,
                        logit_W, logit_b)
    fn, _ = get_compiled()
    out = fn(*args)
    return np.asarray(jax.block_until_ready(out), np.float32)

